# revision 4
# baseline (speedup 1.0000x reference)
"""Trainium2 Bass kernel for causal self-attention (B=2, T=2048, C=1024, H=16).

Sharding: tensor-parallel over heads x data-parallel over batch.
Each of the 8 cores handles one (batch b, head-group g) pair: b = core // 4,
g = core % 4, where a head group is 4 consecutive heads (heads 4g..4g+3).

Per-core pipeline (all matmuls in float32r: full-rate PE, ~1.6e-4 rounding):
  1. QKV projection from host-pre-transposed xT [C, T]:
       qT/kT per head-pair [128, T]  (partitions = 2 heads x 64 dims)
       v per head [128, 16*65]       (T-blocks of 128 on partitions; 65th
                                      column per block is 1.0 -> row sums)
  2. Attention per head in transposed layout: S^T[k, q] = kT.T @ qT blocks,
     exp on ACT straight out of PSUM, causal masking by block structure +
     affine_select on diagonal blocks, then yT[d, q] accumulated as
     v_aug.T @ P^T (row 64 = softmax denominator l).
  3. Normalize: yT *= (1/l) broadcast via a K=1 PE outer product.
  4. Output projection: out[t, cout] partial = yT.T @ Wp rows; partials from
     the 4 head-groups of a batch are summed on the host (the TP all-reduce).
"""

import numpy as np
from contextlib import ExitStack

import concourse.bass as bass
import concourse.tile as tile
from concourse import bacc, mybir
from concourse.bass import ts
from concourse.bass_utils import run_bass_kernel_spmd

F32 = mybir.dt.float32
F32R = mybir.dt.float32r
AF = mybir.ActivationFunctionType
PSUM = bass.MemorySpace.PSUM

B, T, C, H = 2, 2048, 1024, 16
HD = C // H              # 64
HPC = 4                  # heads per core
PAIRS = 2                # head pairs per core
CI = C // 128            # 8 contraction chunks
TB = T // 128            # 16 t-blocks
NQC = T // 512           # 4 q-chunks
N_CORES = 8

MM_DT = F32R             # matmul operand dtype


def _emit(tc, nc, xT_d, wq_d, wk_d, wv_d, wp_d, out_d):
    ctx = ExitStack()
    with ctx:
        pers = ctx.enter_context(tc.tile_pool(name="pers", bufs=1))

        qT = [pers.tile([128, T], MM_DT, name=f"qT{p}") for p in range(PAIRS)]
        kT = [pers.tile([128, T], MM_DT, name=f"kT{p}") for p in range(PAIRS)]
        v_sb = [pers.tile([128, TB * 65], MM_DT, name=f"v{h}") for h in range(HPC)]
        yT = [pers.tile([128, T], MM_DT, name=f"yT{p}") for p in range(PAIRS)]
        wp_sb = pers.tile([128, 2048], MM_DT, name="wp")
        ones_sb = pers.tile([1, 64], MM_DT, name="ones")

        nc.sync.dma_start(wp_sb[:], wp_d[:])
        ones_f = pers.tile([128, 1], F32, name="ones_f")
        nc.gpsimd.memset(ones_f[:], 1.0)
        nc.vector.tensor_copy(ones_sb[:], ones_f[0:1, 0:1].broadcast_to([1, 64]))
        for h in range(HPC):
            # 1.0 into column 64 of every 65-wide t-block (softmax denominator)
            nc.vector.tensor_copy(
                v_sb[h][:].rearrange("p (t c) -> p t c", c=65)[:, :, 64:65],
                ones_f[:].unsqueeze(1).broadcast_to([128, TB, 1]),
            )

        # ---------------- Phase A: QKV projections ----------------
        with (
            tc.tile_pool(name="xt", bufs=1) as pool_xt,
            tc.tile_pool(name="wA", bufs=1) as pool_w,
            tc.tile_pool(name="psQK", bufs=3, space=PSUM) as psQK,
            tc.tile_pool(name="psV", bufs=3, space=PSUM) as psV,
        ):
            xT_sb = [pool_xt.tile([128, T], MM_DT, name=f"xt{ci}") for ci in range(CI)]
            for ci in range(CI):
                nc.sync.dma_start(xT_sb[ci][:], xT_d[ts(ci, 128), :])
            wq_sb = [pool_w.tile([128, 1024], MM_DT, name=f"wq{p}") for p in range(PAIRS)]
            wk_sb = [pool_w.tile([128, 1024], MM_DT, name=f"wk{p}") for p in range(PAIRS)]
            wv_sb = pool_w.tile([128, 2048], MM_DT, name="wv")
            for p in range(PAIRS):
                nc.sync.dma_start(wq_sb[p][:], wq_d[p])
                nc.sync.dma_start(wk_sb[p][:], wk_d[p])
            nc.sync.dma_start(wv_sb[:], wv_d[:])

            for p in range(PAIRS):
                for qc in range(NQC):
                    psq = psQK.tile([128, 512], F32, tag="psqk")
                    for ci in range(CI):
                        nc.tensor.matmul(
                            psq[:], wq_sb[p][:, ts(ci, 128)], xT_sb[ci][:, ts(qc, 512)],
                            start=(ci == 0), stop=(ci == CI - 1),
                        )
                    nc.scalar.copy(qT[p][:, ts(qc, 512)], psq[:])
                for qc in range(NQC):
                    psk = psQK.tile([128, 512], F32, tag="psqk")
                    for ci in range(CI):
                        nc.tensor.matmul(
                            psk[:], wk_sb[p][:, ts(ci, 128)], xT_sb[ci][:, ts(qc, 512)],
                            start=(ci == 0), stop=(ci == CI - 1),
                        )
                    nc.vector.tensor_copy(kT[p][:, ts(qc, 512)], psk[:])

            for tb in range(TB):
                psv = psV.tile([128, 256], F32, tag="psv")
                for ci in range(CI):
                    nc.tensor.matmul(
                        psv[:], xT_sb[ci][:, ts(tb, 128)], wv_sb[:, ts(ci, 256)],
                        start=(ci == 0), stop=(ci == CI - 1),
                    )
                for h in range(HPC):
                    nc.vector.tensor_copy(
                        v_sb[h][:, tb * 65: tb * 65 + 64], psv[:, ts(h, 64)]
                    )

        # ---------------- Phase B: attention ----------------
        with (
            tc.tile_pool(name="psS", bufs=2, space=PSUM) as psS,
            tc.tile_pool(name="psY", bufs=2, space=PSUM) as psY,
            tc.tile_pool(name="psB", bufs=2, space=PSUM) as psB,
            tc.tile_pool(name="pP", bufs=3) as pP,
            tc.tile_pool(name="pN", bufs=3) as pN,
        ):
            for h in range(HPC):
                p, off = h // 2, (h % 2) * 64
                for qc in range(NQC):
                    ypt = psY.tile([128, 512], F32, tag="ypt")
                    nkb = 4 * qc + 4     # causal: k-blocks 0 .. 4*qc+3
                    for kb0 in range(0, nkb, 2):
                        sp = psS.tile([128, 1024], F32, tag="sp")
                        for j in (0, 1):
                            kb = kb0 + j
                            nc.tensor.matmul(
                                sp[:, ts(j, 512)],
                                kT[p][off:off + 64, ts(kb, 128)],
                                qT[p][off:off + 64, ts(qc, 512)],
                                start=True, stop=True,
                            )
                        pt = pP.tile([128, 1024], MM_DT, tag="pt")
                        nc.scalar.activation(pt[:], sp[:], AF.Exp)
                        for j in (0, 1):
                            kb = kb0 + j
                            if kb >= 4 * qc:
                                # diagonal chunk: zero q < k — the fully
                                # invalid cols [0, col) and the diagonal
                                # block triangle, in one affine_select
                                col = kb * 128 - qc * 512
                                w = col + 128
                                nc.gpsimd.affine_select(
                                    out=pt[:, j * 512: j * 512 + w],
                                    in_=pt[:, j * 512: j * 512 + w],
                                    compare_op=mybir.AluOpType.is_ge,
                                    fill=0.0, base=-col,
                                    channel_multiplier=-1, pattern=[[1, w]],
                                )
                        for j in (0, 1):
                            kb = kb0 + j
                            nc.tensor.matmul(
                                ypt[0:65, :],
                                v_sb[h][:, kb * 65:(kb + 1) * 65],
                                pt[:, ts(j, 512)],
                                start=(kb == 0), stop=(kb == nkb - 1),
                            )
                    # normalize: yT = num * (1/l), l broadcast via K=1 outer product
                    l_r = pN.tile([1, 512], MM_DT, tag="lr")
                    with nc.allow_low_precision(reason="f32r rounding of 1/l"):
                        nc.vector.reciprocal(l_r[:], ypt[64:65, :])
                    bl = psB.tile([64, 512], F32, tag="bl")
                    nc.tensor.matmul(bl[:], ones_sb[:], l_r[:], start=True, stop=True)
                    bl_sb = pN.tile([64, 512], F32, tag="blsb")
                    nc.vector.tensor_copy(bl_sb[:], bl[:])
                    nc.vector.tensor_mul(
                        yT[p][off:off + 64, ts(qc, 512)], ypt[0:64, :], bl_sb[:]
                    )

        # ---------------- Phase C: output projection ----------------
        with (
            tc.tile_pool(name="psO", bufs=3, space=PSUM) as psO,
            tc.tile_pool(name="pO", bufs=3) as pO,
        ):
            for tb in range(TB):
                for cc in range(2):
                    po = psO.tile([128, 512], F32, tag="po")
                    for p in range(PAIRS):
                        nc.tensor.matmul(
                            po[:], yT[p][:, ts(tb, 128)],
                            wp_sb[:, p * 1024 + cc * 512: p * 1024 + cc * 512 + 512],
                            start=(p == 0), stop=(p == PAIRS - 1),
                        )
                    ot = pO.tile([128, 512], F32, tag="ot")
                    if (tb * 2 + cc) % 2 == 0:
                        nc.scalar.copy(ot[:], po[:])
                    else:
                        nc.vector.tensor_copy(ot[:], po[:])
                    nc.sync.dma_start(out_d[ts(tb, 128), ts(cc, 512)], ot[:])


_NC_CACHE = None


def _build():
    global _NC_CACHE
    if _NC_CACHE is not None:
        return _NC_CACHE
    nc = bacc.Bacc("TRN2", target_bir_lowering=False, debug=False,
                   num_devices=N_CORES)
    xT_d = nc.dram_tensor("xT", [C, T], MM_DT, kind="ExternalInput")
    wq_d = nc.dram_tensor("wq", [PAIRS, 128, 1024], MM_DT, kind="ExternalInput")
    wk_d = nc.dram_tensor("wk", [PAIRS, 128, 1024], MM_DT, kind="ExternalInput")
    wv_d = nc.dram_tensor("wv", [128, 2048], MM_DT, kind="ExternalInput")
    wp_d = nc.dram_tensor("wp", [128, 2048], MM_DT, kind="ExternalInput")
    out_d = nc.dram_tensor("out", [T, C], F32, kind="ExternalOutput")

    with tile.TileContext(nc) as tc:
        _emit(tc, nc, xT_d, wq_d, wk_d, wv_d, wp_d, out_d)
    nc.compile()
    _NC_CACHE = nc
    return nc


def _pack_pair(m):
    # [1024, 128] -> lhsT chunks layout [128, 8*128]
    return np.ascontiguousarray(
        m.reshape(CI, 128, 128).transpose(1, 0, 2).reshape(128, 1024))


def _in_maps(x, w_attn, w_proj):
    x = np.asarray(x, dtype=np.float32)
    w_attn = np.asarray(w_attn, dtype=np.float32)
    w_proj = np.asarray(w_proj, dtype=np.float32)
    xT = [np.ascontiguousarray(x[b].T) for b in range(B)]
    maps = []
    for core in range(N_CORES):
        b, g = core // HPC, core % HPC
        cols = slice(g * 256, (g + 1) * 256)
        wk_full = w_attn[:, 0 * C:1 * C][:, cols]
        wq_full = w_attn[:, 1 * C:2 * C][:, cols] * np.float32(1.0 / np.sqrt(HD))
        wv_full = w_attn[:, 2 * C:3 * C][:, cols]
        wq_in = np.stack([_pack_pair(wq_full[:, p * 128:(p + 1) * 128])
                          for p in range(PAIRS)])
        wk_in = np.stack([_pack_pair(wk_full[:, p * 128:(p + 1) * 128])
                          for p in range(PAIRS)])
        wv_in = np.ascontiguousarray(
            wv_full.reshape(CI, 128, 256).transpose(1, 0, 2).reshape(128, 2048))
        wp_in = np.ascontiguousarray(
            w_proj[g * 256:(g + 1) * 256, :]
            .reshape(PAIRS, 128, 1024).transpose(1, 0, 2).reshape(128, 2048))
        maps.append({"xT": xT[b], "wq": wq_in, "wk": wk_in,
                     "wv": wv_in, "wp": wp_in})
    return maps


def _assemble(results, b_proj):
    b_proj = np.asarray(b_proj, dtype=np.float32)
    out = np.zeros((B, T, C), dtype=np.float32)
    for core in range(N_CORES):
        out[core // HPC] += results[core]["out"]
    out += b_proj[None, None, :]
    return out


def kernel(x, w_attn, w_proj, b_proj):
    nc = _build()
    maps = _in_maps(x, w_attn, w_proj)
    res = run_bass_kernel_spmd(nc, maps, list(range(N_CORES)))
    return _assemble(res.results, b_proj)


def kernel_traced(x, w_attn, w_proj, b_proj):
    """Like kernel() but with NTFF tracing; returns (out, BassKernelResults)."""
    nc = _build()
    maps = _in_maps(x, w_attn, w_proj)
    res = run_bass_kernel_spmd(nc, maps, list(range(N_CORES)), trace=True)
    return _assemble(res.results, b_proj), res


# revision 8
# speedup vs baseline: 1.0272x; 1.0272x over previous
"""Trainium2 Bass kernel for causal self-attention (B=2, T=2048, C=1024, H=16).

Sharding: tensor-parallel over heads x data-parallel over batch.
Each of the 8 cores handles one (batch b, head-group g) pair: b = core // 4,
g = core % 4, where a head group is 4 consecutive heads (heads 4g..4g+3).

Per-core pipeline:
  1. QKV projection from host-pre-transposed xT [C, T]:
       qT/kT per head-pair [128, T]  (partitions = 2 heads x 64 dims)
       v per head [128, 16*65]       (T-blocks of 128 on partitions; 65th
                                      column per block is 1.0 -> row sums)
  2. Attention per head in transposed layout, k-block outer (weights reused
     across q-chunks): S^T[k, q] = kT.T @ qT with one wide PSUM tile per
     k-block, causal masking via a precomputed -1e9 mask add (DVE) on the
     diagonal chunk, one wide exp (ACT) out of PSUM, then yT[d, q]
     accumulated as v_aug.T @ P^T (row 64 = softmax denominator l).
  3. Normalize: l -> SBUF (ACT), K=1 outer-product broadcast into the same
     PSUM tile's rows 64:128, reciprocal + multiply on DVE.
  4. Output projection: out[t, cout] partial = yT.T @ Wp rows; partials from
     the 4 head-groups of a batch are summed on the host (the TP all-reduce).

Matmul dtype configurable: KBASS_CFG in {f32r, attn_bf16, bf16}.
"""

import os
import numpy as np
from contextlib import ExitStack

import concourse.bass as bass
import concourse.tile as tile
from concourse import bacc, mybir
from concourse.bass import ts
from concourse.bass_utils import run_bass_kernel_spmd

F32 = mybir.dt.float32
F32R = mybir.dt.float32r
BF16 = mybir.dt.bfloat16
AF = mybir.ActivationFunctionType
PSUM = bass.MemorySpace.PSUM

B, T, C, H = 2, 2048, 1024, 16
HD = C // H              # 64
HPC = 4                  # heads per core
PAIRS = 2                # head pairs per core
CI = C // 128            # 8 contraction chunks
TB = T // 128            # 16 t-blocks
NQC = T // 512           # 4 q-chunks
N_CORES = 8

CFG = os.environ.get("KBASS_CFG", "f32r")
if CFG == "bf16":
    IO_DT = BF16          # xT / weights dram+sbuf
    QKV_DT = BF16         # qT/kT/v tiles
    P_DT = BF16           # exp output tiles
    Y_DT = BF16           # normalized yT tiles
elif CFG == "attn_bf16":
    IO_DT = F32R
    QKV_DT = BF16
    P_DT = BF16
    Y_DT = F32R
else:
    IO_DT = F32R
    QKV_DT = F32R
    P_DT = F32R
    Y_DT = F32R


def _emit(tc, nc, xT_d, wq_d, wk_d, wv_d, wp_d, out_d):
    ctx = ExitStack()
    with ctx:
        pers = ctx.enter_context(tc.tile_pool(name="pers", bufs=1))

        qT = [pers.tile([128, T], QKV_DT, name=f"qT{p}") for p in range(PAIRS)]
        kT = [pers.tile([128, T], QKV_DT, name=f"kT{p}") for p in range(PAIRS)]
        v_sb = [pers.tile([128, TB * 65], QKV_DT, name=f"v{h}") for h in range(HPC)]
        yT = [pers.tile([128, T], Y_DT, name=f"yT{p}") for p in range(PAIRS)]
        wp_sb = pers.tile([128, 2048], IO_DT, name="wp")
        ones_sb = pers.tile([1, 64], F32R, name="ones")
        # -1e9 where q-local < k-local + col, else 0; one tile per col offset
        masks = [pers.tile([128, 512], F32, name=f"mask{j}") for j in range(4)]

        nc.sync.dma_start(wp_sb[:], wp_d[:])
        ones_f = pers.tile([128, 1], F32, name="ones_f")
        nc.gpsimd.memset(ones_f[:], 1.0)
        nc.vector.tensor_copy(ones_sb[:], ones_f[0:1, 0:1].broadcast_to([1, 64]))
        for h in range(HPC):
            # 1.0 into column 64 of every 65-wide t-block (softmax denominator)
            nc.vector.tensor_copy(
                v_sb[h][:].rearrange("p (t c) -> p t c", c=65)[:, :, 64:65],
                ones_f[:].unsqueeze(1).broadcast_to([128, TB, 1]),
            )
        for j in range(4):
            nc.gpsimd.memset(masks[j][:], 0.0)
            nc.gpsimd.affine_select(
                out=masks[j][:], in_=masks[j][:],
                compare_op=mybir.AluOpType.is_ge, fill=-1e9,
                base=-j * 128, channel_multiplier=-1, pattern=[[1, 512]],
            )

        # ---------------- Phase A: QKV projections ----------------
        with (
            tc.tile_pool(name="xt", bufs=1) as pool_xt,
            tc.tile_pool(name="wA", bufs=1) as pool_w,
            tc.tile_pool(name="psQK", bufs=3, space=PSUM) as psQK,
            tc.tile_pool(name="psV", bufs=3, space=PSUM) as psV,
        ):
            xT_sb = [pool_xt.tile([128, T], IO_DT, name=f"xt{ci}") for ci in range(CI)]
            for ci in range(CI):
                nc.sync.dma_start(xT_sb[ci][:], xT_d[ts(ci, 128), :])
            wq_sb = [pool_w.tile([128, 1024], IO_DT, name=f"wq{p}") for p in range(PAIRS)]
            wk_sb = [pool_w.tile([128, 1024], IO_DT, name=f"wk{p}") for p in range(PAIRS)]
            wv_sb = pool_w.tile([128, 2048], IO_DT, name="wv")
            for p in range(PAIRS):
                nc.sync.dma_start(wq_sb[p][:], wq_d[p])
                nc.sync.dma_start(wk_sb[p][:], wk_d[p])
            nc.sync.dma_start(wv_sb[:], wv_d[:])

            for p in range(PAIRS):
                for qc in range(NQC):
                    psq = psQK.tile([128, 512], F32, tag="psqk")
                    for ci in range(CI):
                        nc.tensor.matmul(
                            psq[:], wq_sb[p][:, ts(ci, 128)], xT_sb[ci][:, ts(qc, 512)],
                            start=(ci == 0), stop=(ci == CI - 1),
                        )
                    nc.scalar.copy(qT[p][:, ts(qc, 512)], psq[:])
                for qc in range(NQC):
                    psk = psQK.tile([128, 512], F32, tag="psqk")
                    for ci in range(CI):
                        nc.tensor.matmul(
                            psk[:], wk_sb[p][:, ts(ci, 128)], xT_sb[ci][:, ts(qc, 512)],
                            start=(ci == 0), stop=(ci == CI - 1),
                        )
                    nc.vector.tensor_copy(kT[p][:, ts(qc, 512)], psk[:])

            for tb in range(TB):
                psv = psV.tile([128, 256], F32, tag="psv")
                for ci in range(CI):
                    nc.tensor.matmul(
                        psv[:], xT_sb[ci][:, ts(tb, 128)], wv_sb[:, ts(ci, 256)],
                        start=(ci == 0), stop=(ci == CI - 1),
                    )
                for h in range(HPC):
                    nc.vector.tensor_copy(
                        v_sb[h][:, tb * 65: tb * 65 + 64], psv[:, ts(h, 64)]
                    )

        # ---------------- Phase B: attention ----------------
        with (
            tc.tile_pool(name="psS", bufs=2, space=PSUM) as psS,
            tc.tile_pool(name="psY", bufs=2, space=PSUM) as psY,
            tc.tile_pool(name="psB", bufs=2, space=PSUM) as psB,
            tc.tile_pool(name="pP", bufs=3) as pP,
            tc.tile_pool(name="pN", bufs=3) as pN,
        ):
            for h in range(HPC):
                p, off = h // 2, (h % 2) * 64
                for qc in range(NQC):
                    ypt = psY.tile([128, 512], F32, tag="ypt")
                    nkb = 4 * qc + 4     # causal: k-blocks 0 .. 4*qc+3
                    for kb0 in range(0, nkb, 2):
                        sp = psS.tile([128, 1024], F32, tag="sp")
                        for j in (0, 1):
                            kb = kb0 + j
                            nc.tensor.matmul(
                                sp[:, ts(j, 512)],
                                kT[p][off:off + 64, ts(kb, 128)],
                                qT[p][off:off + 64, ts(qc, 512)],
                                start=True, stop=True,
                            )
                        for j in (0, 1):
                            kb = kb0 + j
                            if kb >= 4 * qc:   # diagonal chunk: mask q < k
                                nc.vector.tensor_add(
                                    sp[:, ts(j, 512)], sp[:, ts(j, 512)],
                                    masks[kb - 4 * qc][:],
                                )
                        pt = pP.tile([128, 1024], P_DT, tag="pt")
                        nc.scalar.activation(pt[:], sp[:], AF.Exp)
                        for j in (0, 1):
                            kb = kb0 + j
                            nc.tensor.matmul(
                                ypt[0:65, :],
                                v_sb[h][:, kb * 65:(kb + 1) * 65],
                                pt[:, ts(j, 512)],
                                start=(kb == 0), stop=(kb == nkb - 1),
                            )
                    # normalize: yT = num * (1/l); l broadcast via K=1 outer
                    l_sb = pN.tile([1, 512], F32R, tag="lr")
                    nc.scalar.copy(l_sb[:], ypt[64:65, :])
                    bl = psB.tile([64, 512], F32, tag="bl")
                    nc.tensor.matmul(bl[:], ones_sb[:], l_sb[:],
                                     start=True, stop=True)
                    rl = pN.tile([64, 512], F32, tag="rl")
                    nc.vector.reciprocal(rl[:], bl[:])
                    nc.vector.tensor_mul(
                        yT[p][off:off + 64, ts(qc, 512)], ypt[0:64, :], rl[:]
                    )

        # ---------------- Phase C: output projection ----------------
        with (
            tc.tile_pool(name="psO", bufs=3, space=PSUM) as psO,
            tc.tile_pool(name="pO", bufs=3) as pO,
        ):
            for tb in range(TB):
                for cc in range(2):
                    po = psO.tile([128, 512], F32, tag="po")
                    for p in range(PAIRS):
                        nc.tensor.matmul(
                            po[:], yT[p][:, ts(tb, 128)],
                            wp_sb[:, p * 1024 + cc * 512: p * 1024 + cc * 512 + 512],
                            start=(p == 0), stop=(p == PAIRS - 1),
                        )
                    ot = pO.tile([128, 512], F32, tag="ot")
                    if (tb * 2 + cc) % 2 == 0:
                        nc.scalar.copy(ot[:], po[:])
                    else:
                        nc.vector.tensor_copy(ot[:], po[:])
                    nc.sync.dma_start(out_d[ts(tb, 128), ts(cc, 512)], ot[:])


_NC_CACHE = None


def _build():
    global _NC_CACHE
    if _NC_CACHE is not None:
        return _NC_CACHE
    nc = bacc.Bacc("TRN2", target_bir_lowering=False, debug=False,
                   num_devices=N_CORES)
    xT_d = nc.dram_tensor("xT", [C, T], IO_DT, kind="ExternalInput")
    wq_d = nc.dram_tensor("wq", [PAIRS, 128, 1024], IO_DT, kind="ExternalInput")
    wk_d = nc.dram_tensor("wk", [PAIRS, 128, 1024], IO_DT, kind="ExternalInput")
    wv_d = nc.dram_tensor("wv", [128, 2048], IO_DT, kind="ExternalInput")
    wp_d = nc.dram_tensor("wp", [128, 2048], IO_DT, kind="ExternalInput")
    out_d = nc.dram_tensor("out", [T, C], F32, kind="ExternalOutput")

    with tile.TileContext(nc) as tc:
        _emit(tc, nc, xT_d, wq_d, wk_d, wv_d, wp_d, out_d)
    nc.compile()
    _NC_CACHE = nc
    return nc


def _pack_pair(m):
    # [1024, 128] -> lhsT chunks layout [128, 8*128]
    return np.ascontiguousarray(
        m.reshape(CI, 128, 128).transpose(1, 0, 2).reshape(128, 1024))


def _io_np(a):
    if IO_DT == BF16:
        import ml_dtypes
        return np.ascontiguousarray(a.astype(ml_dtypes.bfloat16))
    return np.ascontiguousarray(a.astype(np.float32))


def _in_maps(x, w_attn, w_proj):
    x = np.asarray(x, dtype=np.float32)
    w_attn = np.asarray(w_attn, dtype=np.float32)
    w_proj = np.asarray(w_proj, dtype=np.float32)
    xT = [_io_np(x[b].T) for b in range(B)]
    maps = []
    for core in range(N_CORES):
        b, g = core // HPC, core % HPC
        cols = slice(g * 256, (g + 1) * 256)
        wk_full = w_attn[:, 0 * C:1 * C][:, cols]
        wq_full = w_attn[:, 1 * C:2 * C][:, cols] * np.float32(1.0 / np.sqrt(HD))
        wv_full = w_attn[:, 2 * C:3 * C][:, cols]
        wq_in = np.stack([_pack_pair(wq_full[:, p * 128:(p + 1) * 128])
                          for p in range(PAIRS)])
        wk_in = np.stack([_pack_pair(wk_full[:, p * 128:(p + 1) * 128])
                          for p in range(PAIRS)])
        wv_in = wv_full.reshape(CI, 128, 256).transpose(1, 0, 2).reshape(128, 2048)
        wp_in = (w_proj[g * 256:(g + 1) * 256, :]
                 .reshape(PAIRS, 128, 1024).transpose(1, 0, 2).reshape(128, 2048))
        maps.append({"xT": xT[b], "wq": _io_np(wq_in), "wk": _io_np(wk_in),
                     "wv": _io_np(wv_in), "wp": _io_np(wp_in)})
    return maps


def _assemble(results, b_proj):
    b_proj = np.asarray(b_proj, dtype=np.float32)
    out = np.zeros((B, T, C), dtype=np.float32)
    for core in range(N_CORES):
        out[core // HPC] += results[core]["out"]
    out += b_proj[None, None, :]
    return out


def kernel(x, w_attn, w_proj, b_proj):
    nc = _build()
    maps = _in_maps(x, w_attn, w_proj)
    res = run_bass_kernel_spmd(nc, maps, list(range(N_CORES)))
    return _assemble(res.results, b_proj)


def kernel_traced(x, w_attn, w_proj, b_proj):
    """Like kernel() but with NTFF tracing; returns (out, BassKernelResults)."""
    nc = _build()
    maps = _in_maps(x, w_attn, w_proj)
    res = run_bass_kernel_spmd(nc, maps, list(range(N_CORES)), trace=True)
    return _assemble(res.results, b_proj), res


# revision 12
# speedup vs baseline: 1.4451x; 1.4069x over previous
"""Trainium2 Bass kernel for causal self-attention (B=2, T=2048, C=1024, H=16).

Sharding: tensor-parallel over heads x data-parallel over batch.
Each of the 8 cores handles one (batch b, head-group g) pair: b = core // 4,
g = core % 4, where a head group is 4 consecutive heads (heads 4g..4g+3).

Per-core pipeline:
  1. QKV projection from host-pre-transposed xT [C, T]:
       qT/kT per head-pair [128, T]  (partitions = 2 heads x 64 dims)
       v per head [128, 16*65]       (T-blocks of 128 on partitions; 65th
                                      column per block is 1.0 -> row sums)
  2. Attention per head in transposed layout: S^T[k, q] = kT.T @ qT blocks.
     The two heads of a pair run as interleaved chains whose S matmuls sit
     in different PE row groups (base partitions 0/64) and execute
     concurrently in the array. exp on ACT straight out of PSUM; causal
     masking by 0/1 mask multiplies on DVE; yT[d, q] accumulated as
     v_aug.T @ P^T (row 64 = softmax denominator l).
  3. Normalize: l -> SBUF (ACT), partition-broadcast on GPSIMD,
     reciprocal_approx_fast + multiply on DVE.
  4. Output projection: out[t, cout] partial = yT.T @ Wp rows; partials from
     the 4 head-groups of a batch are summed on the host (the TP all-reduce).

All SBUF pools stay open for the whole kernel (everything fits), so phases
overlap freely; only PSUM pools are scoped.

Matmul dtype configurable: KBASS_CFG in {f32r, attn_bf16, bf16}.
"""

import os
import numpy as np
from contextlib import ExitStack

import concourse.bass as bass
import concourse.tile as tile
from concourse import bacc, library_config, mybir
from concourse.bass import ts
from concourse.bass_utils import run_bass_kernel_spmd

F32 = mybir.dt.float32
F32R = mybir.dt.float32r
BF16 = mybir.dt.bfloat16
AF = mybir.ActivationFunctionType
PSUM = bass.MemorySpace.PSUM

B, T, C, H = 2, 2048, 1024, 16
HD = C // H              # 64
HPC = 4                  # heads per core
PAIRS = 2                # head pairs per core
CI = C // 128            # 8 contraction chunks
TB = T // 128            # 16 t-blocks
NQC = T // 512           # 4 q-chunks
N_CORES = 8

CFG = os.environ.get("KBASS_CFG", "f32r")
if CFG == "bf16":
    IO_DT = BF16          # xT / weights dram+sbuf
    QKV_DT = BF16         # qT/kT/v tiles
    P_DT = BF16           # exp output tiles
    Y_DT = BF16           # normalized yT tiles
elif CFG == "attn_bf16":
    IO_DT = F32R
    QKV_DT = BF16
    P_DT = BF16
    Y_DT = F32R
else:
    IO_DT = F32R
    QKV_DT = F32R
    P_DT = F32R
    Y_DT = F32R

# 1/l broadcast: gpsimd partition_broadcast (default) or PE outer product
GP_BCAST = os.environ.get("KBASS_GP_BCAST", "1") == "1"


def _emit(tc, nc, xT_d, wq_d, wk_d, wv_d, wp_d, out_d):
    ctx = ExitStack()
    with ctx:
        pers = ctx.enter_context(tc.tile_pool(name="pers", bufs=1))
        if GP_BCAST:
            nc.gpsimd.load_library(library_config.attn)

        qT = [pers.tile([128, T], QKV_DT, name=f"qT{p}") for p in range(PAIRS)]
        kT = [pers.tile([128, T], QKV_DT, name=f"kT{p}") for p in range(PAIRS)]
        v_sb = [pers.tile([128, TB * 65], QKV_DT, name=f"v{h}") for h in range(HPC)]
        yT = [pers.tile([128, T], Y_DT, name=f"yT{p}") for p in range(PAIRS)]
        wp_sb = pers.tile([128, 2048], IO_DT, name="wp")
        ones_sb = pers.tile([1, 64], F32R, name="ones")
        # 0/1 causal masks: mask01[j][x, y] = (y - x - 128j >= 0), [128, 512]
        masks = [pers.tile([128, 512], P_DT, name=f"mask{j}") for j in range(4)]

        nc.sync.dma_start(wp_sb[:], wp_d[:])
        ones_f = pers.tile([128, 1], F32, name="ones_f")
        nc.gpsimd.memset(ones_f[:], 1.0)
        nc.vector.tensor_copy(ones_sb[:], ones_f[0:1, 0:1].broadcast_to([1, 64]))
        for h in range(HPC):
            # 1.0 into column 64 of every 65-wide t-block (softmax denominator)
            nc.vector.tensor_copy(
                v_sb[h][:].rearrange("p (t c) -> p t c", c=65)[:, :, 64:65],
                ones_f[:].unsqueeze(1).broadcast_to([128, TB, 1]),
            )
        mask_f = pers.tile([128, 512], F32, name="mask_f")
        for j in range(4):
            nc.gpsimd.memset(mask_f[:], 1.0)
            nc.gpsimd.affine_select(
                out=mask_f[:], in_=mask_f[:],
                compare_op=mybir.AluOpType.is_ge, fill=0.0,
                base=-j * 128, channel_multiplier=-1, pattern=[[1, 512]],
            )
            nc.vector.tensor_copy(masks[j][:], mask_f[:])

        xT_sb = [pers.tile([128, T], IO_DT, name=f"xt{ci}") for ci in range(CI)]
        for ci in range(CI):
            nc.sync.dma_start(xT_sb[ci][:], xT_d[ts(ci, 128), :])
        wq_sb = [pers.tile([128, 1024], IO_DT, name=f"wq{p}") for p in range(PAIRS)]
        wk_sb = [pers.tile([128, 1024], IO_DT, name=f"wk{p}") for p in range(PAIRS)]
        wv_sb = pers.tile([128, 2048], IO_DT, name="wv")
        for p in range(PAIRS):
            nc.sync.dma_start(wq_sb[p][:], wq_d[p])
            nc.sync.dma_start(wk_sb[p][:], wk_d[p])
        nc.sync.dma_start(wv_sb[:], wv_d[:])

        # ---------------- Phase A: QKV projections ----------------
        with (
            tc.tile_pool(name="psQK", bufs=3, space=PSUM) as psQK,
            tc.tile_pool(name="psV", bufs=3, space=PSUM) as psV,
        ):
            for p in range(PAIRS):
                for qc in range(NQC):
                    psq = psQK.tile([128, 512], F32, tag="psqk")
                    for ci in range(CI):
                        nc.tensor.matmul(
                            psq[:], wq_sb[p][:, ts(ci, 128)], xT_sb[ci][:, ts(qc, 512)],
                            start=(ci == 0), stop=(ci == CI - 1),
                        )
                    nc.vector.tensor_copy(qT[p][:, ts(qc, 512)], psq[:])
                for qc in range(NQC):
                    psk = psQK.tile([128, 512], F32, tag="psqk")
                    for ci in range(CI):
                        nc.tensor.matmul(
                            psk[:], wk_sb[p][:, ts(ci, 128)], xT_sb[ci][:, ts(qc, 512)],
                            start=(ci == 0), stop=(ci == CI - 1),
                        )
                    nc.vector.tensor_copy(kT[p][:, ts(qc, 512)], psk[:])

            for tb in range(TB):
                psv = psV.tile([128, 256], F32, tag="psv")
                for ci in range(CI):
                    nc.tensor.matmul(
                        psv[:], xT_sb[ci][:, ts(tb, 128)], wv_sb[:, ts(ci, 256)],
                        start=(ci == 0), stop=(ci == CI - 1),
                    )
                for h in range(HPC):
                    nc.vector.tensor_copy(
                        v_sb[h][:, tb * 65: tb * 65 + 64], psv[:, ts(h, 64)]
                    )

        # ---------------- Phase B: attention ----------------
        with (
            tc.tile_pool(name="psS", bufs=1, space=PSUM) as psS,
            tc.tile_pool(name="psY", bufs=2, space=PSUM) as psY,
            tc.tile_pool(name="pP", bufs=3) as pP,
            tc.tile_pool(name="pN", bufs=3) as pN,
        ):
            for p in range(PAIRS):
                for qc in range(NQC):
                    ypt2 = [psY.tile([128, 512], F32, tag=f"ypt{hh}",
                                     name=f"ypt_p{p}q{qc}h{hh}") for hh in (0, 1)]
                    nkb = 4 * qc + 4     # causal: k-blocks 0 .. 4*qc+3
                    for kb0 in range(0, nkb, 2):
                        sps = [psS.tile([128, 1024], F32, tag=f"sp{hh}",
                                        name=f"sp{hh}") for hh in (0, 1)]
                        # S matmuls: heads alternate row groups -> concurrent
                        for j in (0, 1):
                            kb = kb0 + j
                            for hh in (0, 1):
                                off = hh * 64
                                nc.tensor.matmul(
                                    sps[hh][:, ts(j, 512)],
                                    kT[p][off:off + 64, ts(kb, 128)],
                                    qT[p][off:off + 64, ts(qc, 512)],
                                    start=True, stop=True,
                                )
                        pts = []
                        for hh in (0, 1):
                            pt = pP.tile([128, 1024], P_DT, tag=f"pt{hh}",
                                         name=f"pt{hh}")
                            nc.scalar.activation(pt[:], sps[hh][:], AF.Exp)
                            for j in (0, 1):
                                kb = kb0 + j
                                if kb >= 4 * qc:   # diagonal chunk: zero q < k
                                    nc.vector.tensor_mul(
                                        pt[:, ts(j, 512)], pt[:, ts(j, 512)],
                                        masks[kb - 4 * qc][:],
                                    )
                            pts.append(pt)
                        for j in (0, 1):
                            kb = kb0 + j
                            for hh in (0, 1):
                                nc.tensor.matmul(
                                    ypt2[hh][0:65, :],
                                    v_sb[2 * p + hh][:, kb * 65:(kb + 1) * 65],
                                    pts[hh][:, ts(j, 512)],
                                    start=(kb == 0), stop=(kb == nkb - 1),
                                )
                    # normalize: yT = num * (1/l)
                    for hh in (0, 1):
                        off = hh * 64
                        if GP_BCAST:
                            l_sb = pN.tile([1, 512], F32, tag="lr")
                            nc.scalar.copy(l_sb[:], ypt2[hh][64:65, :])
                            lb = pN.tile([64, 512], F32, tag="lb")
                            nc.gpsimd.partition_broadcast(lb[:], l_sb[:])
                            rl = pN.tile([64, 512], F32, tag="rl")
                            nc.vector.reciprocal_approx_fast(rl[:], lb[:])
                        else:
                            l_sb = pN.tile([1, 512], F32R, tag="lr")
                            nc.scalar.copy(l_sb[:], ypt2[hh][64:65, :])
                            bl = psY.tile([64, 512], F32, tag="bl")
                            nc.tensor.matmul(bl[:], ones_sb[:], l_sb[:],
                                             start=True, stop=True)
                            rl = pN.tile([64, 512], F32, tag="rl")
                            nc.vector.reciprocal_approx_fast(rl[:], bl[:])
                        nc.vector.tensor_mul(
                            yT[p][off:off + 64, ts(qc, 512)],
                            ypt2[hh][0:64, :], rl[:],
                        )

        # ---------------- Phase C: output projection ----------------
        with (
            tc.tile_pool(name="psO", bufs=3, space=PSUM) as psO,
            tc.tile_pool(name="pO", bufs=3) as pO,
        ):
            for tb in range(TB):
                for cc in range(2):
                    po = psO.tile([128, 512], F32, tag="po")
                    for p in range(PAIRS):
                        nc.tensor.matmul(
                            po[:], yT[p][:, ts(tb, 128)],
                            wp_sb[:, p * 1024 + cc * 512: p * 1024 + cc * 512 + 512],
                            start=(p == 0), stop=(p == PAIRS - 1),
                        )
                    ot = pO.tile([128, 512], F32, tag="ot")
                    nc.vector.tensor_copy(ot[:], po[:])
                    nc.sync.dma_start(out_d[ts(tb, 128), ts(cc, 512)], ot[:])


_NC_CACHE = None


def _build():
    global _NC_CACHE
    if _NC_CACHE is not None:
        return _NC_CACHE
    nc = bacc.Bacc("TRN2", target_bir_lowering=False, debug=False,
                   num_devices=N_CORES)
    xT_d = nc.dram_tensor("xT", [C, T], IO_DT, kind="ExternalInput")
    wq_d = nc.dram_tensor("wq", [PAIRS, 128, 1024], IO_DT, kind="ExternalInput")
    wk_d = nc.dram_tensor("wk", [PAIRS, 128, 1024], IO_DT, kind="ExternalInput")
    wv_d = nc.dram_tensor("wv", [128, 2048], IO_DT, kind="ExternalInput")
    wp_d = nc.dram_tensor("wp", [128, 2048], IO_DT, kind="ExternalInput")
    out_d = nc.dram_tensor("out", [T, C], F32, kind="ExternalOutput")

    with tile.TileContext(nc) as tc:
        _emit(tc, nc, xT_d, wq_d, wk_d, wv_d, wp_d, out_d)
    nc.compile()
    _NC_CACHE = nc
    return nc


def _pack_pair(m):
    # [1024, 128] -> lhsT chunks layout [128, 8*128]
    return np.ascontiguousarray(
        m.reshape(CI, 128, 128).transpose(1, 0, 2).reshape(128, 1024))


def _io_np(a):
    if IO_DT == BF16:
        import ml_dtypes
        return np.ascontiguousarray(a.astype(ml_dtypes.bfloat16))
    return np.ascontiguousarray(a.astype(np.float32))


def _in_maps(x, w_attn, w_proj):
    x = np.asarray(x, dtype=np.float32)
    w_attn = np.asarray(w_attn, dtype=np.float32)
    w_proj = np.asarray(w_proj, dtype=np.float32)
    xT = [_io_np(x[b].T) for b in range(B)]
    maps = []
    for core in range(N_CORES):
        b, g = core // HPC, core % HPC
        cols = slice(g * 256, (g + 1) * 256)
        wk_full = w_attn[:, 0 * C:1 * C][:, cols]
        wq_full = w_attn[:, 1 * C:2 * C][:, cols] * np.float32(1.0 / np.sqrt(HD))
        wv_full = w_attn[:, 2 * C:3 * C][:, cols]
        wq_in = np.stack([_pack_pair(wq_full[:, p * 128:(p + 1) * 128])
                          for p in range(PAIRS)])
        wk_in = np.stack([_pack_pair(wk_full[:, p * 128:(p + 1) * 128])
                          for p in range(PAIRS)])
        wv_in = wv_full.reshape(CI, 128, 256).transpose(1, 0, 2).reshape(128, 2048)
        wp_in = (w_proj[g * 256:(g + 1) * 256, :]
                 .reshape(PAIRS, 128, 1024).transpose(1, 0, 2).reshape(128, 2048))
        maps.append({"xT": xT[b], "wq": _io_np(wq_in), "wk": _io_np(wk_in),
                     "wv": _io_np(wv_in), "wp": _io_np(wp_in)})
    return maps


def _assemble(results, b_proj):
    b_proj = np.asarray(b_proj, dtype=np.float32)
    out = np.zeros((B, T, C), dtype=np.float32)
    for core in range(N_CORES):
        out[core // HPC] += results[core]["out"]
    out += b_proj[None, None, :]
    return out


def kernel(x, w_attn, w_proj, b_proj):
    nc = _build()
    maps = _in_maps(x, w_attn, w_proj)
    res = run_bass_kernel_spmd(nc, maps, list(range(N_CORES)))
    return _assemble(res.results, b_proj)


def kernel_traced(x, w_attn, w_proj, b_proj):
    """Like kernel() but with NTFF tracing; returns (out, BassKernelResults)."""
    nc = _build()
    maps = _in_maps(x, w_attn, w_proj)
    res = run_bass_kernel_spmd(nc, maps, list(range(N_CORES)), trace=True)
    return _assemble(res.results, b_proj), res
